# revision 1
# baseline (speedup 1.0000x reference)
"""Causal self-attention (dense transformer block) on 8 trn2 NeuronCores.

Sharding: tensor-parallel over heads. Each core owns 2 of the 16 heads:
  - qkv projection: column-slice of W_qkv (128 cols per core)
  - attention for its (2 heads x 2 batches) = 4 (b,h) pairs
  - out projection: row-slice of W_out -> partial y [4096, 1024]
Host sums the 8 partial y's and adds b_out (+ the v-bias term folded
through W_out, since softmax rows sum to 1).

Device pipeline (bf16 matmuls, fp32 accumulation), interleaved per
(batch, 512-row block) so PE/ACT/DVE/GPSIMD overlap:
  proj block:  qT,kT[hd, s] = W.T @ xT (+bias, DVE); v via PE-transpose
               of vT -> v_aug [v_h0 | 1 | v_h1 | 1]
  attn block:  per k-tile: sT[k, q] both heads side by side in one PSUM
               tile, one exp (ScalarE, no max subtraction: scores are
               bounded), causal mask via one gpsimd affine_select on
               diagonal tiles, PV: attnT[65, q] += v_aug.T @ PT
               (row 64 = softmax denominators), normalize via
               reciprocal_approx_fast + partition_broadcast,
  out-proj:    y[q, e] = attnT_tile.T @ W_out_rows, evict, store.
"""

import sys

if "/opt/trn_rl_repo" not in sys.path:
    sys.path.insert(0, "/opt/trn_rl_repo")

import numpy as np
import ml_dtypes

import concourse.bass as bass
import concourse.tile as tile
from concourse import bacc, mybir
from concourse.bass_utils import run_bass_kernel_spmd
from concourse.masks import make_identity

BF16 = mybir.dt.bfloat16
F32 = mybir.dt.float32
AF = mybir.ActivationFunctionType

N_EMBED = 1024
N_HEAD = 16
HEAD_DIM = 64
N_CORES = 8
HEADS_PER_CORE = N_HEAD // N_CORES          # 2
DCORE = HEADS_PER_CORE * HEAD_DIM           # 128 head-dims per core
B = 2
S = 2048                                    # seq len per batch
QB = 512                                    # q-block (moving free dim)
KT = 128                                    # k-tile (contraction tile)
DT = N_EMBED // 128                         # 8 d-tiles for projections
SCALE = 1.0 / 8.0                           # 1/sqrt(HEAD_DIM)
VW = HEAD_DIM + 1                           # v_aug slice width per head
H = HEADS_PER_CORE


def build_program(seq=S):
    """Build the per-core Bass program (identical on all cores; SPMD)."""
    s_tot = B * seq                 # total rows across batches
    n_qb = seq // QB                # q-blocks per batch
    n_kt = seq // KT                # k-tiles per batch
    kt_per_qb = QB // KT            # 4

    nc = bacc.Bacc("TRN2", target_bir_lowering=False, debug=False,
                   num_devices=N_CORES)

    xT = nc.dram_tensor("xT", [N_EMBED, s_tot], BF16, kind="ExternalInput")
    wq = nc.dram_tensor("wq", [N_EMBED, DCORE], BF16, kind="ExternalInput")
    wk = nc.dram_tensor("wk", [N_EMBED, DCORE], BF16, kind="ExternalInput")
    wv = nc.dram_tensor("wv", [N_EMBED, DCORE], BF16, kind="ExternalInput")
    bq = nc.dram_tensor("bq", [DCORE, 1], F32, kind="ExternalInput")
    bk = nc.dram_tensor("bk", [DCORE, 1], F32, kind="ExternalInput")
    wout = nc.dram_tensor("wout", [DCORE, N_EMBED], BF16, kind="ExternalInput")
    y = nc.dram_tensor("y", [s_tot, N_EMBED], BF16, kind="ExternalOutput")

    xT_r = xT.ap().rearrange("(t p) s -> p t s", p=128)

    with (
        tile.TileContext(nc) as tc,
        tc.tile_pool(name="singles", bufs=1) as singles,
        # PSUM (8 banks): sy 2x[128,1024]=4, attn 2x[65,512]=2, pv 2x1=2
        tc.tile_pool(name="sy_ps", bufs=2, space="PSUM") as sy_pool,
        tc.tile_pool(name="attn_ps", bufs=1, space="PSUM") as attn_pool,
        tc.tile_pool(name="pv_ps", bufs=2, space="PSUM") as pv_pool,
        tc.tile_pool(name="vstage", bufs=3) as vstage_pool,
        tc.tile_pool(name="pt_sb", bufs=6) as pt_pool,
        tc.tile_pool(name="rec_sb", bufs=3) as rec_pool,
        tc.tile_pool(name="bc_sb", bufs=3) as bc_pool,
        tc.tile_pool(name="at_sb", bufs=4) as at_pool,
        tc.tile_pool(name="y_sb", bufs=6) as ysb_pool,
    ):
        # ---- persistent SBUF tensors ----
        xT_sb = singles.tile([128, DT, s_tot], BF16)
        wq_sb = singles.tile([128, DT, DCORE], BF16)
        wk_sb = singles.tile([128, DT, DCORE], BF16)
        wv_sb = singles.tile([128, DT, DCORE], BF16)
        bq_sb = singles.tile([DCORE, 1], F32)
        bk_sb = singles.tile([DCORE, 1], F32)
        wout_sb = singles.tile([DCORE, N_EMBED], BF16)
        qT_sb = singles.tile([DCORE, s_tot], BF16)
        kT_sb = singles.tile([DCORE, s_tot], BF16)
        # v_aug: per global k-tile kt: [v_h0 | 1 | v_h1 | 1]
        v_aug = singles.tile([128, B * n_kt, 2 * VW], BF16)
        ident_sb = singles.tile([128, 128], BF16)

        # ---- input DMAs (xT split per d-tile so compute can start) ----
        nc.sync.dma_start(out=wq_sb,
                          in_=wq.ap().rearrange("(t p) h -> p t h", p=128))
        nc.sync.dma_start(out=wk_sb,
                          in_=wk.ap().rearrange("(t p) h -> p t h", p=128))
        nc.sync.dma_start(out=wv_sb,
                          in_=wv.ap().rearrange("(t p) h -> p t h", p=128))
        nc.sync.dma_start(out=bq_sb, in_=bq.ap())
        nc.sync.dma_start(out=bk_sb, in_=bk.ap())
        nc.sync.dma_start(out=wout_sb, in_=wout.ap())
        # column-wise: s-block sb's full-depth slice arrives together,
        # so proj/attention of block 0 start ~1MB into the load
        for sb in range(s_tot // QB):
            nc.sync.dma_start(out=xT_sb[:, :, sb * QB:(sb + 1) * QB],
                              in_=xT_r[:, :, sb * QB:(sb + 1) * QB])

        make_identity(nc, ident_sb)
        nc.vector.memset(v_aug[:, :, HEAD_DIM], 1.0)
        nc.vector.memset(v_aug[:, :, 2 * HEAD_DIM + 1], 1.0)

        def proj_block(sb):
            """Projections for 512-row block sb (global)."""
            sl = slice(sb * QB, (sb + 1) * QB)
            for w_sb, b_sb, dst in ((wq_sb, bq_sb, qT_sb),
                                    (wk_sb, bk_sb, kT_sb)):
                ps = pv_pool.tile([128, QB], F32, tag="aux", name="proj")
                for t in range(DT):
                    nc.tensor.matmul(ps, lhsT=w_sb[:, t, :],
                                     rhs=xT_sb[:, t, sl],
                                     start=(t == 0), stop=(t == DT - 1))
                nc.vector.tensor_scalar_add(dst[:, sl], ps, b_sb)
            ps = pv_pool.tile([128, QB], F32, tag="aux", name="proj")
            for t in range(DT):
                nc.tensor.matmul(ps, lhsT=wv_sb[:, t, :],
                                 rhs=xT_sb[:, t, sl],
                                 start=(t == 0), stop=(t == DT - 1))
            vstage = vstage_pool.tile([128, QB], BF16)
            nc.vector.tensor_copy(vstage, ps)
            for u in range(QB // 128):
                kt_gl = (QB // 128) * sb + u
                tr = pv_pool.tile([128, 128], BF16, tag="aux", name="tr")
                nc.tensor.transpose(tr, vstage[:, u * 128:(u + 1) * 128],
                                    ident_sb)
                nc.vector.tensor_copy(v_aug[:, kt_gl, 0:HEAD_DIM],
                                      tr[:, 0:HEAD_DIM])
                nc.vector.tensor_copy(
                    v_aug[:, kt_gl, HEAD_DIM + 1:2 * HEAD_DIM + 1],
                    tr[:, HEAD_DIM:2 * HEAD_DIM])

        def attn_kloop(b_i, j):
            """Score/exp/mask/PV loop for q-block j of batch b_i.

            Diagonal k-tiles first so the gpsimd masks run while the
            off-diagonal matmuls proceed. Returns evicted (at64, r0)
            SBUF tiles per head for the deferred normalization."""
            qsl = slice(b_i * seq + j * QB, b_i * seq + (j + 1) * QB)
            attn_ps = [attn_pool.tile([VW, QB], F32, tag=f"attn{h}",
                                      name=f"attn{h}") for h in range(H)]
            kts = list(range(kt_per_qb * j, kt_per_qb * (j + 1))) + \
                list(range(0, kt_per_qb * j))

            def emit_pv(kt, pt, off, pos):
                for h in range(H):
                    nc.tensor.matmul(
                        attn_ps[h][:, off:],
                        lhsT=v_aug[:, b_i * n_kt + kt, VW * h:VW * (h + 1)],
                        rhs=pt[:, h, off:],
                        start=(pos == 0), stop=(pos == len(kts) - 1))

            prev = None
            for pos, kt in enumerate(kts):
                ks = slice(b_i * seq + kt * 128, b_i * seq + kt * 128 + 128)
                d = kt - kt_per_qb * j
                off = 128 * d if d >= 0 else 0   # first valid q column
                s_ps = sy_pool.tile([128, H, QB], F32, tag="sy", name="s_ps")
                pt = pt_pool.tile([128, H, QB], BF16, tag="pt", name="pt")
                for h in range(H):
                    hsl = slice(HEAD_DIM * h, HEAD_DIM * (h + 1))
                    nc.tensor.matmul(
                        s_ps[:, h, off:],
                        lhsT=kT_sb[hsl, ks],
                        rhs=qT_sb[hsl, qsl.start + off:qsl.stop],
                        start=True, stop=True)
                nc.scalar.activation(pt[:, :, off:], s_ps[:, :, off:],
                                     AF.Exp, scale=SCALE)
                if d >= 0:  # diagonal: mask both heads at once
                    nc.gpsimd.affine_select(
                        out=pt[:, :, off:], in_=pt[:, :, off:],
                        compare_op=mybir.AluOpType.is_ge, fill=0.0,
                        base=0, channel_multiplier=-1,
                        pattern=[[0, H], [1, QB - off]])
                # PV deferred one k-tile: scores(kt+1) runs on PE while
                # exp(kt) is still on ScalarE
                if prev is not None:
                    emit_pv(*prev)
                prev = (kt, pt, off, pos)
            emit_pv(*prev)
            # evict accumulators to SBUF to free the PSUM banks
            evicted = []
            for h in range(H):
                at64 = at_pool.tile([HEAD_DIM, QB], F32, tag=f"at64{h}",
                                    name=f"at64{h}")
                nc.vector.tensor_copy(at64, attn_ps[h][0:HEAD_DIM, :])
                r0 = rec_pool.tile([1, QB], F32, tag=f"r0{h}", name=f"r0{h}")
                nc.vector.tensor_copy(r0, attn_ps[h][HEAD_DIM:HEAD_DIM + 1, :])
                evicted.append((at64, r0))
            return evicted

        def norm_outproj(b_i, j, evicted):
            """Deferred normalization + out-projection for q-block j."""
            at_bj = at_pool.tile([DCORE, QB], BF16, name="at_bj")
            for h, (at64, r0) in enumerate(evicted):
                rf = rec_pool.tile([1, QB], F32, tag=f"rf{h}", name=f"rf{h}")
                nc.vector.reciprocal_approx_fast(rf, r0)
                bc_sb = bc_pool.tile([HEAD_DIM, QB], F32, tag=f"bc{h}",
                                     name=f"bc{h}")
                nc.gpsimd.partition_broadcast(bc_sb, rf)
                nc.vector.tensor_mul(
                    at_bj[HEAD_DIM * h:HEAD_DIM * (h + 1), :], at64, bc_sb)
            for qt in range(QB // 128):
                at = at_bj[:, qt * 128:(qt + 1) * 128]
                ysb = ysb_pool.tile([128, N_EMBED], BF16, tag="ysb",
                                    name="ysb")
                for u in range(N_EMBED // QB):
                    yp = pv_pool.tile([128, QB], F32, tag="aux", name="yp")
                    nc.tensor.matmul(yp, lhsT=at,
                                     rhs=wout_sb[:, u * QB:(u + 1) * QB],
                                     start=True, stop=True)
                    if u == 0:
                        nc.vector.tensor_copy(ysb[:, 0:QB], yp)
                    else:
                        nc.scalar.copy(ysb[:, QB:2 * QB], yp)
                row0 = b_i * seq + j * QB + qt * 128
                nc.sync.dma_start(out=y.ap()[row0:row0 + 128, :], in_=ysb)

        # ---- interleaved schedule: proj frontloaded 3 blocks ahead,
        # norm/out-proj one block behind ----
        n_blocks = B * n_qb
        next_proj = 0
        for _ in range(3):
            if next_proj < n_blocks:
                proj_block(next_proj)
                next_proj += 1
        pending = None
        for b_i in range(B):
            for j in range(n_qb):
                if next_proj < n_blocks:
                    proj_block(next_proj)
                    next_proj += 1
                evicted = attn_kloop(b_i, j)
                if pending is not None:
                    norm_outproj(*pending)
                pending = (b_i, j, evicted)
        norm_outproj(*pending)

    nc.compile()
    return nc


_CACHE = {}


def _get_program(seq=S):
    if seq not in _CACHE:
        _CACHE[seq] = build_program(seq)
    return _CACHE[seq]


def make_in_maps(x, W_qkv, b_qkv, seq=S):
    bf16 = ml_dtypes.bfloat16
    s_tot = B * seq
    xT = np.ascontiguousarray(
        x.reshape(s_tot, N_EMBED).T).astype(bf16)
    in_maps = []
    for c in range(N_CORES):
        csl = slice(DCORE * c, DCORE * (c + 1))
        in_maps.append({
            "xT": xT,
            "wq": np.ascontiguousarray(W_qkv[:, csl]).astype(bf16),
            "wk": np.ascontiguousarray(W_qkv[:, N_EMBED:][:, csl]).astype(bf16),
            "wv": np.ascontiguousarray(W_qkv[:, 2 * N_EMBED:][:, csl]).astype(bf16),
            "bq": np.ascontiguousarray(
                b_qkv[csl].reshape(DCORE, 1)).astype(np.float32),
            "bk": np.ascontiguousarray(
                b_qkv[N_EMBED:][csl].reshape(DCORE, 1)).astype(np.float32),
            "wout": None,  # filled by caller
        })
    return in_maps


def kernel(x, W_qkv, b_qkv, W_out, b_out):
    x = np.asarray(x, dtype=np.float32)
    W_qkv = np.asarray(W_qkv, dtype=np.float32)
    b_qkv = np.asarray(b_qkv, dtype=np.float32)
    W_out = np.asarray(W_out, dtype=np.float32)
    b_out = np.asarray(b_out, dtype=np.float32)

    nc = _get_program(S)
    in_maps = make_in_maps(x, W_qkv, b_qkv, S)
    bf16 = ml_dtypes.bfloat16
    for c in range(N_CORES):
        csl = slice(DCORE * c, DCORE * (c + 1))
        in_maps[c]["wout"] = np.ascontiguousarray(W_out[csl, :]).astype(bf16)

    res = run_bass_kernel_spmd(nc, in_maps, core_ids=list(range(N_CORES)))
    y = np.zeros((B * S, N_EMBED), dtype=np.float32)
    for r in res.results:
        y += r["y"].astype(np.float32)
    # bias + v-bias folded through W_out (softmax rows sum to 1)
    y += b_out[None, :] + b_qkv[2 * N_EMBED:] @ W_out
    return y.reshape(B, S, N_EMBED)



# revision 68
# speedup vs baseline: 1.2592x; 1.2592x over previous
"""Causal self-attention (dense transformer block) on 8 trn2 NeuronCores.

Sharding: tensor-parallel over heads. Each core owns 2 of the 16 heads:
  - qkv projection: column-slice of W_qkv (128 cols per core)
  - attention for its (2 heads x 2 batches) = 4 (b,h) pairs
  - out projection: row-slice of W_out -> partial y [4096, 1024]
Host sums the 8 partial y's and adds b_out (+ the v-bias term folded
through W_out, since softmax rows sum to 1).

Device schedule (bf16 matmuls, fp32 accumulation): a software-pipelined
attention k-loop with "filler" interleaving.  Per k-tile iteration the
PE emits scores(kt+1), a few filler matmuls (projection steps for
future blocks / out-projection of past blocks), then PV(kt-2).  The
2-tile PV deferral hides the ScalarE exp (and gpsimd causal mask on
diagonal tiles); the fillers keep the PE saturated (and in its fast
p-state) while ACT chews through exp, instead of the proj/out-proj
bursts that used to stall both engines.

V is projected directly in [token, head-dim] layout (x-chunk stationary)
so no PE transpose / vstage copy is needed; the PSUM->SBUF eviction
writes the v_aug [v_h0 | 1 | v_h1 | 1] layout whose extra ones-columns
make the PV matmul accumulate softmax denominators in row 64.
"""

import collections
import math
import sys

if "/opt/trn_rl_repo" not in sys.path:
    sys.path.insert(0, "/opt/trn_rl_repo")

import numpy as np
import ml_dtypes

import concourse.bass as bass
import concourse.tile as tile
from concourse import bacc, mybir
from concourse.bass_utils import run_bass_kernel_spmd

BF16 = mybir.dt.bfloat16
F32 = mybir.dt.float32
FP8 = mybir.dt.float8e4
AF = mybir.ActivationFunctionType

N_EMBED = 1024
N_HEAD = 16
HEAD_DIM = 64
N_CORES = 8
HEADS_PER_CORE = N_HEAD // N_CORES          # 2
DCORE = HEADS_PER_CORE * HEAD_DIM           # 128 head-dims per core
B = 2
S = 2048                                    # seq len per batch
QB = 512                                    # q-block (moving free dim)
KT = 128                                    # k-tile (contraction tile)
DT = N_EMBED // 128                         # 8 d-tiles for projections
SCALE = 1.0 / 8.0                           # 1/sqrt(HEAD_DIM)
VW = HEAD_DIM + 1                           # v_aug slice width per head
H = HEADS_PER_CORE


def build_program(seq=S):
    """Build the per-core Bass program (identical on all cores; SPMD)."""
    s_tot = B * seq                 # total rows across batches
    n_qb = seq // QB                # q-blocks per batch
    n_kt = seq // KT                # k-tiles per batch
    kt_per_qb = QB // KT            # 4
    n_blocks = B * n_qb             # 8 token blocks of 512

    nc = bacc.Bacc("TRN2", target_bir_lowering=False, debug=False,
                   num_devices=N_CORES)

    NP = DT // 2                    # d-tile pairs for fp8 DoubleRow

    # Projections run on the fp8 DoubleRow path (256-deep contraction at
    # 2x rate): x and the qkv weights are split hi/lo into e4m3 on the host
    # (weights pre-scaled x64 to clear the e4m3 subnormal floor; the scale
    # is folded into the exp argument and the softmax denominators).
    # 3 terms wh*xh + wh*xl + wl*xh ~= w*x to ~0.1% -- better than bf16.
    xh = nc.dram_tensor("xh", [128, NP, 2, s_tot], FP8, kind="ExternalInput")
    xl = nc.dram_tensor("xl", [128, NP, 2, s_tot], FP8, kind="ExternalInput")
    wqh = nc.dram_tensor("wqh", [128, NP, 2, DCORE], FP8, kind="ExternalInput")
    wql = nc.dram_tensor("wql", [128, NP, 2, DCORE], FP8, kind="ExternalInput")
    wkh = nc.dram_tensor("wkh", [128, NP, 2, DCORE], FP8, kind="ExternalInput")
    wkl = nc.dram_tensor("wkl", [128, NP, 2, DCORE], FP8, kind="ExternalInput")
    wvh = nc.dram_tensor("wvh", [128, NP, 2, DCORE], FP8, kind="ExternalInput")
    wvl = nc.dram_tensor("wvl", [128, NP, 2, DCORE], FP8, kind="ExternalInput")
    bq = nc.dram_tensor("bq", [DCORE, 1], F32, kind="ExternalInput")
    bk = nc.dram_tensor("bk", [DCORE, 1], F32, kind="ExternalInput")
    wout = nc.dram_tensor("wout", [DCORE, N_EMBED], BF16, kind="ExternalInput")
    # causal mask for diagonal k-tiles: cm[p, h, q] = 1.0 if q >= p else 0.0
    # (only the first 128 columns of a diagonal tile can be masked)
    cm = nc.dram_tensor("cm", [128, H, KT], BF16, kind="ExternalInput")
    y = nc.dram_tensor("y", [s_tot, N_EMBED], BF16, kind="ExternalOutput")

    with (
        tile.TileContext(nc) as tc,
        tc.tile_pool(name="singles", bufs=1) as singles,
        # PSUM (8 banks): sy 2x[128,2,512]=4, attn 2x[65,512]=2, aux 2x1=2
        tc.tile_pool(name="sy_ps", bufs=2, space="PSUM") as sy_pool,
        tc.tile_pool(name="attn_ps", bufs=1, space="PSUM") as attn_pool,
        tc.tile_pool(name="aux_ps", bufs=2, space="PSUM") as aux_pool,
        tc.tile_pool(name="pt_sb", bufs=6) as pt_pool,
        tc.tile_pool(name="ev_sb", bufs=2) as ev_pool,
        tc.tile_pool(name="rec_sb", bufs=2) as rec_pool,
        tc.tile_pool(name="bc_sb", bufs=2) as bc_pool,
        tc.tile_pool(name="at_sb", bufs=2) as at_pool,
        tc.tile_pool(name="y_sb", bufs=4) as ysb_pool,
    ):
        # ---- persistent SBUF tensors ----
        xh_sb = singles.tile([128, NP, 2, s_tot], FP8)
        xl_sb = singles.tile([128, NP, 2, s_tot], FP8)
        wqh_sb = singles.tile([128, NP, 2, DCORE], FP8)
        wql_sb = singles.tile([128, NP, 2, DCORE], FP8)
        wkh_sb = singles.tile([128, NP, 2, DCORE], FP8)
        wkl_sb = singles.tile([128, NP, 2, DCORE], FP8)
        wvh_sb = singles.tile([128, NP, 2, DCORE], FP8)
        wvl_sb = singles.tile([128, NP, 2, DCORE], FP8)
        bq_sb = singles.tile([DCORE, 1], F32)
        bk_sb = singles.tile([DCORE, 1], F32)
        wout_sb = singles.tile([DCORE, N_EMBED], BF16)
        cm_sb = singles.tile([128, H, KT], BF16)
        qT_sb = singles.tile([DCORE, s_tot], BF16)
        kT_sb = singles.tile([DCORE, s_tot], BF16)
        # v_aug per global k-tile: [v_h0 | 1 | v_h1 | 1]
        v_aug = singles.tile([128, B * n_kt, 2 * VW], BF16)

        # ---- input DMAs, ordered so block-0 compute starts ASAP ----
        def xslice(sb):
            sl = slice(sb * QB, (sb + 1) * QB)
            nc.sync.dma_start(out=xh_sb[:, :, :, sl], in_=xh.ap()[:, :, :, sl])
            nc.sync.dma_start(out=xl_sb[:, :, :, sl], in_=xl.ap()[:, :, :, sl])

        # block 0's hi-x arrives per d-tile-pair so the first projection
        # steps stream as the data lands
        nc.sync.dma_start(out=wqh_sb, in_=wqh.ap())
        nc.gpsimd.dma_start(out=wkh_sb, in_=wkh.ap())
        for tp in range(NP):
            q = nc.sync if tp % 2 == 0 else nc.gpsimd
            q.dma_start(out=xh_sb[:, tp, :, 0:QB],
                        in_=xh.ap()[:, tp, :, 0:QB])
        for tp in range(NP):
            q = nc.gpsimd if tp % 2 == 0 else nc.sync
            q.dma_start(out=xl_sb[:, tp, :, 0:QB],
                        in_=xl.ap()[:, tp, :, 0:QB])
        nc.sync.dma_start(out=wql_sb, in_=wql.ap())
        nc.gpsimd.dma_start(out=wkl_sb, in_=wkl.ap())
        nc.sync.dma_start(out=bq_sb, in_=bq.ap())
        nc.gpsimd.dma_start(out=bk_sb, in_=bk.ap())
        nc.sync.dma_start(out=wvh_sb, in_=wvh.ap())
        nc.sync.dma_start(out=wvl_sb, in_=wvl.ap())
        nc.sync.dma_start(out=cm_sb, in_=cm.ap())
        xslice(1)
        xslice(2)
        nc.sync.dma_start(out=wout_sb, in_=wout.ap())
        for sb in range(3, n_blocks):
            xslice(sb)

        nc.vector.memset(v_aug[:, :, HEAD_DIM], 1.0)
        nc.vector.memset(v_aug[:, :, VW + HEAD_DIM], 1.0)

        DR = mybir.MatmulPerfMode.DoubleRow
        NQK = 3 * NP                # DoubleRow steps per q/k projection

        def qk_step(wh_sb, wl_sb, b_sb, dst, sb, st, cell):
            """One fp8 DoubleRow step of the q or k projection of block sb.
            Steps 0..NP-1: wh*xh, NP..2NP-1: wh*xl, 2NP..3NP-1: wl*xh."""
            sl = slice(sb * QB, (sb + 1) * QB)
            term, tp = divmod(st, NP)
            w_t = (wh_sb, wh_sb, wl_sb)[term]
            x_t = (xh_sb, xl_sb, xh_sb)[term]
            if st == 0:
                cell["ps"] = aux_pool.tile([128, QB], F32, tag="aux",
                                           name="pps")
            nc.tensor.matmul(cell["ps"], lhsT=w_t[:, tp],
                             rhs=x_t[:, tp, :, sl], perf_mode=DR,
                             start=(st == 0), stop=(st == NQK - 1))
            if st == NQK - 1:
                nc.vector.tensor_scalar_add(dst[:, sl], cell["ps"], b_sb)

        def v_chunk(sb, c, cell):
            """V projection for token chunk c of block sb (fp8 DoubleRow,
            direct [token, head-dim] layout); evicts into v_aug."""
            tsl = slice(sb * QB + c * 128, sb * QB + (c + 1) * 128)
            if c == 0:
                cell["ps"] = aux_pool.tile([128, 4, 128], F32, tag="aux",
                                           name="vps")
            ps = cell["ps"]
            for st in range(NQK):
                term, tp = divmod(st, NP)
                x_t = (xh_sb, xl_sb, xh_sb)[term]
                w_t = (wvh_sb, wvh_sb, wvl_sb)[term]
                nc.tensor.matmul(ps[:, c, :], lhsT=x_t[:, tp, :, tsl],
                                 rhs=w_t[:, tp], perf_mode=DR,
                                 start=(st == 0), stop=(st == NQK - 1))
            kt_gl = sb * kt_per_qb + c
            # one strided copy fills both heads' v columns (strides 65/64)
            dst = v_aug[:, kt_gl, 0:2 * VW].rearrange(
                "p (a b) -> p a b", a=2, b=VW)[:, :, 0:HEAD_DIM]
            srcv = ps[:, c, :].rearrange("p (a b) -> p a b", a=2, b=HEAD_DIM)
            nc.vector.tensor_copy(dst, srcv)

        def proj_fillers(sb):
            units = []
            cq, ck, cv = {}, {}, {}
            for st2 in range(NQK // 2):
                units.append(lambda s=st2: (
                    qk_step(wqh_sb, wql_sb, bq_sb, qT_sb, sb, 2 * s, cq),
                    qk_step(wqh_sb, wql_sb, bq_sb, qT_sb, sb, 2 * s + 1, cq)))
            for st2 in range(NQK // 2):
                units.append(lambda s=st2: (
                    qk_step(wkh_sb, wkl_sb, bk_sb, kT_sb, sb, 2 * s, ck),
                    qk_step(wkh_sb, wkl_sb, bk_sb, kT_sb, sb, 2 * s + 1, ck)))
            for c in range(kt_per_qb):
                units.append(lambda c=c: v_chunk(sb, c, cv))
            return [("proj", sb, u) for u in units]

        ysb_cell = {}

        def yp_unit(b_i, j, at_bj, qt, u):
            """Out-projection: one [128 q, 512 e] matmul + evict; both
            halves land in one [128, 1024] staging tile, stored by a single
            DMA (halving the HWDGE dispatch load)."""
            at = at_bj[:, qt * 128:(qt + 1) * 128]
            yp = aux_pool.tile([128, QB], F32, tag="aux", name="yp")
            nc.tensor.matmul(yp, lhsT=at, rhs=wout_sb[:, u * QB:(u + 1) * QB],
                             start=True, stop=True)
            if u == 0:
                ysb_cell["t"] = ysb_pool.tile([128, N_EMBED], BF16,
                                              tag="ysb", name="ysb")
            ysb = ysb_cell["t"]
            # gpsimd cannot read PSUM: evictions go to DVE / ACT only
            if u == 0:
                nc.vector.tensor_copy(ysb[:, 0:QB], yp)
            else:
                nc.scalar.copy(ysb[:, QB:], yp)
                row0 = b_i * seq + j * QB + qt * 128
                nc.sync.dma_start(out=y.ap()[row0:row0 + 128, :], in_=ysb)

        def yp_fillers(b_i, j, at_bj):
            return [("yp", None,
                     lambda qt=qt, u=u: yp_unit(b_i, j, at_bj, qt, u))
                    for qt in range(QB // 128) for u in range(N_EMBED // QB)]

        fillers = collections.deque()

        def filler_tick(n, no_yp=False):
            """Emit up to n non-out-proj fillers plus at most one out-proj
            unit.  The out-proj unit rides on top of the budget: its ACT/DVE
            eviction copy delays the exp stream, so the iteration must
            stretch by extra PE work to keep the score pipeline fed."""
            popped_yp = 0
            for _ in range(min(n, len(fillers))):
                if fillers[0][0] == "yp":
                    if no_yp or popped_yp >= 1:
                        break
                    popped_yp += 1
                fillers.popleft()[2]()

        # ---- attention block ----
        def attn_block(b_i, j, f, late=(), eager=None, chunk_pv=False):
            """Software-pipelined k-loop for q-block j of batch b_i.

            Scores emitted diagonal-tiles-first (causal mask on DVE via the
            constant triangular mask); PV trails scores by 2 k-tiles; f
            fillers per iter; `late` fillers join the queue at iteration 2
            (previous block's out-proj, gated behind its norm chain)."""
            qsl = slice(b_i * seq + j * QB, b_i * seq + (j + 1) * QB)
            attn_ps = [attn_pool.tile([VW, QB], F32, tag=f"attn{h}",
                                      name=f"attn{h}") for h in range(H)]
            kts = list(range(kt_per_qb * j, kt_per_qb * (j + 1))) + \
                list(range(0, kt_per_qb * j))
            K = len(kts)
            saved = {}

            def emit_scores(kt):
                ks = slice(b_i * seq + kt * 128, b_i * seq + kt * 128 + 128)
                d = kt - kt_per_qb * j
                off = 128 * d if d >= 0 else 0
                s_ps = sy_pool.tile([128, H, QB], F32, tag="sy", name="s_ps")
                pt = pt_pool.tile([128, H, QB], BF16, tag="pt", name="pt")
                for h in range(H):
                    hsl = slice(HEAD_DIM * h, HEAD_DIM * (h + 1))
                    nc.tensor.matmul(
                        s_ps[:, h, off:],
                        lhsT=kT_sb[hsl, ks],
                        rhs=qT_sb[hsl, qsl.start + off:qsl.stop],
                        start=True, stop=True)
                nc.scalar.activation(pt[:, :, off:], s_ps[:, :, off:],
                                     AF.Exp, scale=SCALE / 4096.0)
                if d >= 0:  # diagonal: mask the 128 cols that need it
                    nc.vector.tensor_mul(pt[:, :, off:off + KT],
                                         pt[:, :, off:off + KT], cm_sb)
                saved[kt] = (pt, off)

            def emit_pv(kt, pos):
                if chunk_pv:
                    # j=0 block: accumulate column chunk #pos completely
                    # (k-tiles 0..pos back-to-back, a well-formed PSUM
                    # group) so it can normalize mid-epilogue
                    c = pos
                    for h in range(H):
                        for kt2 in range(c + 1):
                            pt, _ = saved[kt2]
                            nc.tensor.matmul(
                                attn_ps[h][:, c * 128:(c + 1) * 128],
                                lhsT=v_aug[:, b_i * n_kt + kt2,
                                           VW * h:VW * (h + 1)],
                                rhs=pt[:, h, c * 128:(c + 1) * 128],
                                start=(kt2 == 0), stop=(kt2 == c))
                    return
                pt, off = saved.pop(kt)
                for h in range(H):
                    nc.tensor.matmul(
                        attn_ps[h][:, off:],
                        lhsT=v_aug[:, b_i * n_kt + kt, VW * h:VW * (h + 1)],
                        rhs=pt[:, h, off:],
                        start=(pos == 0), stop=(pos == K - 1))

            filler_tick(3)
            emit_scores(kts[0])
            for t in range(K):
                if t + 1 < K:
                    emit_scores(kts[t + 1])
                if t == 2:
                    fillers.extend(late)
                filler_tick(f)
                if t - 2 >= 0:
                    emit_pv(kts[t - 2], t - 2)
                    if eager:
                        eager(attn_ps, t - 2)
            if K <= 2:
                fillers.extend(late)
            filler_tick(2)
            emit_pv(kts[K - 2], K - 2)
            if eager:
                eager(attn_ps, K - 2)
            filler_tick(2)
            emit_pv(kts[K - 1], K - 1)
            if eager:
                eager(attn_ps, K - 1)
            filler_tick(2)
            return attn_ps

        def norm_block(attn_ps):
            """Normalize the block's attention accumulators straight out of
            PSUM: reciprocal of the denominator row, broadcast, and a fused
            multiply-evict into the bf16 out-proj operand."""
            at_bj = at_pool.tile([DCORE, QB], BF16, name="at_bj")
            rfs, bcs = [], []
            for h in range(H):
                r0 = rec_pool.tile([1, QB], F32, tag=f"r0{h}", name=f"r0{h}")
                # x64 undoes the weight pre-scale carried by v through PV
                nc.vector.tensor_scalar_mul(
                    r0, attn_ps[h][HEAD_DIM:HEAD_DIM + 1, :], 64.0)
                rf = rec_pool.tile([1, QB], F32, tag=f"rf{h}", name=f"rf{h}")
                nc.vector.reciprocal_approx_fast(rf, r0)
                rfs.append(rf)
            for h in range(H):
                bc = bc_pool.tile([HEAD_DIM, QB], F32, tag=f"bc{h}",
                                  name=f"bc{h}")
                nc.gpsimd.partition_broadcast(bc, rfs[h])
                bcs.append(bc)
            for h in range(H):
                nc.vector.tensor_mul(
                    at_bj[HEAD_DIM * h:HEAD_DIM * (h + 1), :],
                    attn_ps[h][0:HEAD_DIM, :], bcs[h])
            return at_bj

        rot = [lambda o, i: nc.vector.tensor_copy(o, i),
               lambda o, i: nc.scalar.copy(o, i)]
        rot_i = [0]
        fin_cell = {"left": 0}

        def yp_unit_fin(at, row0, u):
            """Drain-phase out-proj unit: PSUM from the (now idle) score
            banks, eviction copies alternating DVE/ACT."""
            if fin_cell["left"] == 0:
                fin_cell["t"] = sy_pool.tile([128, H, QB], F32,
                                             tag="sy", name="ypw")
                fin_cell["left"] = H
            yp = fin_cell["t"][:, H - fin_cell["left"], :]
            fin_cell["left"] -= 1
            nc.tensor.matmul(yp, lhsT=at,
                             rhs=wout_sb[:, u * QB:(u + 1) * QB],
                             start=True, stop=True)
            if u == 0:
                fin_cell["ysb"] = ysb_pool.tile([128, N_EMBED], BF16,
                                                tag="ysb", name="ysb")
            ysb = fin_cell["ysb"]
            rot[rot_i[0] % 2](ysb[:, u * QB:(u + 1) * QB], yp)
            rot_i[0] += 1
            if u == 1:
                nc.sync.dma_start(out=y.ap()[row0:row0 + 128, :], in_=ysb)

        def norm_chunk(attn_ps, c, atqs):
            """Per-128-column normalize of a closed accumulator chunk."""
            csl = slice(c * 128, (c + 1) * 128)
            atq = at_pool.tile([DCORE, 128], BF16, tag="atq", name="atq")
            for h in range(H):
                r0 = rec_pool.tile([1, 128], F32, tag=f"r0c{h}",
                                   name=f"r0c{h}")
                nc.vector.tensor_scalar_mul(
                    r0, attn_ps[h][HEAD_DIM:HEAD_DIM + 1, csl], 64.0)
                rf = rec_pool.tile([1, 128], F32, tag=f"rfc{h}",
                                   name=f"rfc{h}")
                nc.vector.reciprocal_approx_fast(rf, r0)
                bc = bc_pool.tile([HEAD_DIM, 128], F32, tag=f"bcc{h}",
                                  name=f"bcc{h}")
                nc.gpsimd.partition_broadcast(bc, rf)
                nc.vector.tensor_mul(
                    atq[HEAD_DIM * h:HEAD_DIM * (h + 1), :],
                    attn_ps[h][0:HEAD_DIM, csl], bc)
            atqs[c] = atq

        # ---- schedule ----
        # attention order: small (b1, j0) last to shrink the tail
        order = [(0, 0), (0, 1), (0, 2), (0, 3),
                 (1, 1), (1, 2), (1, 3), (1, 0)]
        # proj groups to enqueue as fillers at each attention block start
        enqueue = {0: [1, 2], 1: [3], 2: [4], 3: [5], 4: [6], 5: [7]}

        # block 0 q/k projection is the critical path: emit directly,
        # q/k steps interleaved per d-tile pair to match the DMA feed rate
        cq0, ck0, cv0 = {}, {}, {}
        for st in range(NQK):
            qk_step(wqh_sb, wql_sb, bq_sb, qT_sb, 0, st, cq0)
            qk_step(wkh_sb, wkl_sb, bk_sb, kT_sb, 0, st, ck0)
        for c in range(kt_per_qb):
            fillers.append(("proj", 0, lambda c=c: v_chunk(0, c, cv0)))

        pending = None              # (b_i, j, attn_ps) awaiting norm + yp
        for idx, (b_i, j) in enumerate(order):
            for sb in enqueue.get(idx, []):
                fillers.extend(proj_fillers(sb))
            # fillers needed before the NEXT block can stream:
            # everything up to the last proj/v unit of its token block
            if idx + 1 < len(order):
                nb, nj = order[idx + 1]
                need_sb = {4 * nb + jj for jj in range(nj + 1)}
                needed = 0
                for i, (kind, sb, _) in enumerate(fillers):
                    if kind == "proj" and sb in need_sb:
                        needed = i + 1
            else:
                needed = len(fillers)
            K = 4 * (j + 1)
            f = max(3, math.ceil(needed / K))

            late = ()
            if pending is not None:
                at_prev = norm_block(pending[2])
                late = yp_fillers(pending[0], pending[1], at_prev)
            last = idx == len(order) - 1
            if last:
                # pipeline the last block's norm + out-proj inside its own
                # attention epilogue: chunk c closes at PV #c, normalizes
                # immediately, out-projects one PV later
                atqs = {}
                base = b_i * seq + j * QB

                def eager(aps, pos):
                    norm_chunk(aps, pos, atqs)
                    if pos >= 1:
                        for u in range(N_EMBED // QB):
                            yp_unit_fin(atqs[pos - 1],
                                        base + (pos - 1) * 128, u)
                attn_ps = attn_block(b_i, j, f, late, eager, chunk_pv=True)
                for u in range(N_EMBED // QB):
                    yp_unit_fin(atqs[kt_per_qb - 1],
                                base + (kt_per_qb - 1) * 128, u)
            else:
                attn_ps = attn_block(b_i, j, f, late)
            pending = (b_i, j, attn_ps)

        while fillers:
            filler_tick(1)

    nc.compile()
    return nc


_CACHE = {}


def _get_program(seq=S):
    if seq not in _CACHE:
        _CACHE[seq] = build_program(seq)
    return _CACHE[seq]


WSCALE = 64.0  # qkv weight pre-scale: clears the e4m3 subnormal floor


def make_in_maps(x, W_qkv, b_qkv, seq=S):
    bf16 = ml_dtypes.bfloat16
    fp8 = ml_dtypes.float8_e4m3fn
    s_tot = B * seq
    NP = DT // 2

    # x split hi/lo in e4m3, laid out [128, NP, 2, s_tot]:
    # embed index e = tp*256 + pair*128 + p
    xf = x.reshape(s_tot, N_EMBED)
    x_hi = xf.astype(fp8)
    x_lo = (xf - x_hi.astype(np.float32)).astype(fp8)

    def xarr(xv):
        return np.ascontiguousarray(
            xv.T.reshape(NP, 2, 128, s_tot).transpose(2, 0, 1, 3))

    def wsplit(w):                    # [1024, DCORE] -> hi/lo [128,NP,2,DCORE]
        ws = w * WSCALE
        wh = ws.astype(fp8)
        wl = (ws - wh.astype(np.float32)).astype(fp8)

        def arr(wv):
            return np.ascontiguousarray(
                wv.reshape(NP, 2, 128, DCORE).transpose(2, 0, 1, 3))
        return arr(wh), arr(wl)

    # causal mask for diagonal k-tiles (same for both heads)
    cmask = (np.arange(KT)[None, :] >= np.arange(128)[:, None])
    cmask = np.ascontiguousarray(
        np.broadcast_to(cmask[:, None, :], (128, H, KT))).astype(bf16)

    xh_a, xl_a = xarr(x_hi), xarr(x_lo)
    in_maps = []
    for c in range(N_CORES):
        csl = slice(DCORE * c, DCORE * (c + 1))
        wqh_a, wql_a = wsplit(W_qkv[:, csl])
        wkh_a, wkl_a = wsplit(W_qkv[:, N_EMBED:][:, csl])
        wvh_a, wvl_a = wsplit(W_qkv[:, 2 * N_EMBED:][:, csl])
        in_maps.append({
            "xh": xh_a, "xl": xl_a,
            "wqh": wqh_a, "wql": wql_a,
            "wkh": wkh_a, "wkl": wkl_a,
            "wvh": wvh_a, "wvl": wvl_a,
            "bq": np.ascontiguousarray(
                (b_qkv[csl] * WSCALE).reshape(DCORE, 1)).astype(np.float32),
            "bk": np.ascontiguousarray(
                (b_qkv[N_EMBED:][csl] * WSCALE)
                .reshape(DCORE, 1)).astype(np.float32),
            "cm": cmask,
            "wout": None,  # filled by caller
        })
    return in_maps


def kernel(x, W_qkv, b_qkv, W_out, b_out):
    x = np.asarray(x, dtype=np.float32)
    W_qkv = np.asarray(W_qkv, dtype=np.float32)
    b_qkv = np.asarray(b_qkv, dtype=np.float32)
    W_out = np.asarray(W_out, dtype=np.float32)
    b_out = np.asarray(b_out, dtype=np.float32)

    nc = _get_program(S)
    in_maps = make_in_maps(x, W_qkv, b_qkv, S)
    bf16 = ml_dtypes.bfloat16
    for c in range(N_CORES):
        csl = slice(DCORE * c, DCORE * (c + 1))
        in_maps[c]["wout"] = np.ascontiguousarray(W_out[csl, :]).astype(bf16)

    res = run_bass_kernel_spmd(nc, in_maps, core_ids=list(range(N_CORES)))
    y = np.zeros((B * S, N_EMBED), dtype=np.float32)
    for r in res.results:
        y += r["y"].astype(np.float32)
    # bias + v-bias folded through W_out (softmax rows sum to 1)
    y += b_out[None, :] + b_qkv[2 * N_EMBED:] @ W_out
    return y.reshape(B, S, N_EMBED)


# revision 73
# speedup vs baseline: 1.2597x; 1.0004x over previous
"""Causal self-attention (dense transformer block) on 8 trn2 NeuronCores.

Sharding: tensor-parallel over heads. Each core owns 2 of the 16 heads:
  - qkv projection: column-slice of W_qkv (128 cols per core)
  - attention for its (2 heads x 2 batches) = 4 (b,h) pairs
  - out projection: row-slice of W_out -> partial y [4096, 1024]
Host sums the 8 partial y's and adds b_out (+ the v-bias term folded
through W_out, since softmax rows sum to 1).

Device schedule (bf16 matmuls, fp32 accumulation): a software-pipelined
attention k-loop with "filler" interleaving.  Per k-tile iteration the
PE emits scores(kt+1), a few filler matmuls (projection steps for
future blocks / out-projection of past blocks), then PV(kt-2).  The
2-tile PV deferral hides the ScalarE exp (and gpsimd causal mask on
diagonal tiles); the fillers keep the PE saturated (and in its fast
p-state) while ACT chews through exp, instead of the proj/out-proj
bursts that used to stall both engines.

V is projected directly in [token, head-dim] layout (x-chunk stationary)
so no PE transpose / vstage copy is needed; the PSUM->SBUF eviction
writes the v_aug [v_h0 | 1 | v_h1 | 1] layout whose extra ones-columns
make the PV matmul accumulate softmax denominators in row 64.
"""

import collections
import math
import sys

if "/opt/trn_rl_repo" not in sys.path:
    sys.path.insert(0, "/opt/trn_rl_repo")

import numpy as np
import ml_dtypes

import concourse.bass as bass
import concourse.tile as tile
from concourse import bacc, mybir
from concourse.bass_utils import run_bass_kernel_spmd

BF16 = mybir.dt.bfloat16
F32 = mybir.dt.float32
FP8 = mybir.dt.float8e4
AF = mybir.ActivationFunctionType

N_EMBED = 1024
N_HEAD = 16
HEAD_DIM = 64
N_CORES = 8
HEADS_PER_CORE = N_HEAD // N_CORES          # 2
DCORE = HEADS_PER_CORE * HEAD_DIM           # 128 head-dims per core
B = 2
S = 2048                                    # seq len per batch
QB = 512                                    # q-block (moving free dim)
KT = 128                                    # k-tile (contraction tile)
DT = N_EMBED // 128                         # 8 d-tiles for projections
SCALE = 1.0 / 8.0                           # 1/sqrt(HEAD_DIM)
VW = HEAD_DIM + 1                           # v_aug slice width per head
H = HEADS_PER_CORE


def build_program(seq=S):
    """Build the per-core Bass program (identical on all cores; SPMD)."""
    s_tot = B * seq                 # total rows across batches
    n_qb = seq // QB                # q-blocks per batch
    n_kt = seq // KT                # k-tiles per batch
    kt_per_qb = QB // KT            # 4
    n_blocks = B * n_qb             # 8 token blocks of 512

    nc = bacc.Bacc("TRN2", target_bir_lowering=False, debug=False,
                   num_devices=N_CORES)

    NP = DT // 2                    # d-tile pairs for fp8 DoubleRow

    # Projections run on the fp8 DoubleRow path (256-deep contraction at
    # 2x rate): x and the qkv weights are split hi/lo into e4m3 on the host
    # (weights pre-scaled x64 to clear the e4m3 subnormal floor; the scale
    # is folded into the exp argument and the softmax denominators).
    # 3 terms wh*xh + wh*xl + wl*xh ~= w*x to ~0.1% -- better than bf16.
    xh = nc.dram_tensor("xh", [128, NP, 2, s_tot], FP8, kind="ExternalInput")
    xl = nc.dram_tensor("xl", [128, NP, 2, s_tot], FP8, kind="ExternalInput")
    wqh = nc.dram_tensor("wqh", [128, NP, 2, DCORE], FP8, kind="ExternalInput")
    wql = nc.dram_tensor("wql", [128, NP, 2, DCORE], FP8, kind="ExternalInput")
    wkh = nc.dram_tensor("wkh", [128, NP, 2, DCORE], FP8, kind="ExternalInput")
    wkl = nc.dram_tensor("wkl", [128, NP, 2, DCORE], FP8, kind="ExternalInput")
    wvh = nc.dram_tensor("wvh", [128, NP, 2, DCORE], FP8, kind="ExternalInput")
    wvl = nc.dram_tensor("wvl", [128, NP, 2, DCORE], FP8, kind="ExternalInput")
    bq = nc.dram_tensor("bq", [DCORE, 1], F32, kind="ExternalInput")
    bk = nc.dram_tensor("bk", [DCORE, 1], F32, kind="ExternalInput")
    wout = nc.dram_tensor("wout", [DCORE, N_EMBED], BF16, kind="ExternalInput")
    # causal mask for diagonal k-tiles: cm[p, h, q] = 1.0 if q >= p else 0.0
    # (only the first 128 columns of a diagonal tile can be masked)
    cm = nc.dram_tensor("cm", [128, H, KT], BF16, kind="ExternalInput")
    y = nc.dram_tensor("y", [s_tot, N_EMBED], BF16, kind="ExternalOutput")

    with (
        tile.TileContext(nc) as tc,
        tc.tile_pool(name="singles", bufs=1) as singles,
        # PSUM (8 banks): sy 2x[128,2,512]=4, attn 2x[65,512]=2, aux 2x1=2
        tc.tile_pool(name="sy_ps", bufs=2, space="PSUM") as sy_pool,
        tc.tile_pool(name="attn_ps", bufs=1, space="PSUM") as attn_pool,
        tc.tile_pool(name="aux_ps", bufs=2, space="PSUM") as aux_pool,
        tc.tile_pool(name="pt_sb", bufs=6) as pt_pool,
        tc.tile_pool(name="ev_sb", bufs=2) as ev_pool,
        tc.tile_pool(name="rec_sb", bufs=2) as rec_pool,
        tc.tile_pool(name="bc_sb", bufs=2) as bc_pool,
        tc.tile_pool(name="at_sb", bufs=2) as at_pool,
        tc.tile_pool(name="y_sb", bufs=6) as ysb_pool,
    ):
        # ---- persistent SBUF tensors ----
        xh_sb = singles.tile([128, NP, 2, s_tot], FP8)
        xl_sb = singles.tile([128, NP, 2, s_tot], FP8)
        wqh_sb = singles.tile([128, NP, 2, DCORE], FP8)
        wql_sb = singles.tile([128, NP, 2, DCORE], FP8)
        wkh_sb = singles.tile([128, NP, 2, DCORE], FP8)
        wkl_sb = singles.tile([128, NP, 2, DCORE], FP8)
        wvh_sb = singles.tile([128, NP, 2, DCORE], FP8)
        wvl_sb = singles.tile([128, NP, 2, DCORE], FP8)
        bq_sb = singles.tile([DCORE, 1], F32)
        bk_sb = singles.tile([DCORE, 1], F32)
        wout_sb = singles.tile([DCORE, N_EMBED], BF16)
        cm_sb = singles.tile([128, H, KT], BF16)
        qT_sb = singles.tile([DCORE, s_tot], BF16)
        kT_sb = singles.tile([DCORE, s_tot], BF16)
        # v_aug per global k-tile: [v_h0 | 1 | v_h1 | 1]
        v_aug = singles.tile([128, B * n_kt, 2 * VW], BF16)

        # ---- input DMAs, ordered so block-0 compute starts ASAP ----
        def xslice(sb):
            sl = slice(sb * QB, (sb + 1) * QB)
            nc.sync.dma_start(out=xh_sb[:, :, :, sl], in_=xh.ap()[:, :, :, sl])
            nc.sync.dma_start(out=xl_sb[:, :, :, sl], in_=xl.ap()[:, :, :, sl])

        # block 0's hi-x arrives per d-tile-pair so the first projection
        # steps stream as the data lands
        nc.sync.dma_start(out=wqh_sb, in_=wqh.ap())
        nc.gpsimd.dma_start(out=wkh_sb, in_=wkh.ap())
        for tp in range(NP):
            q = nc.sync if tp % 2 == 0 else nc.gpsimd
            q.dma_start(out=xh_sb[:, tp, :, 0:QB],
                        in_=xh.ap()[:, tp, :, 0:QB])
        for tp in range(NP):
            q = nc.gpsimd if tp % 2 == 0 else nc.sync
            q.dma_start(out=xl_sb[:, tp, :, 0:QB],
                        in_=xl.ap()[:, tp, :, 0:QB])
        nc.sync.dma_start(out=wql_sb, in_=wql.ap())
        nc.gpsimd.dma_start(out=wkl_sb, in_=wkl.ap())
        nc.sync.dma_start(out=bq_sb, in_=bq.ap())
        nc.gpsimd.dma_start(out=bk_sb, in_=bk.ap())
        nc.sync.dma_start(out=wvh_sb, in_=wvh.ap())
        nc.sync.dma_start(out=wvl_sb, in_=wvl.ap())
        nc.sync.dma_start(out=cm_sb, in_=cm.ap())
        xslice(1)
        xslice(2)
        nc.sync.dma_start(out=wout_sb, in_=wout.ap())
        for sb in range(3, n_blocks):
            xslice(sb)

        nc.vector.memset(v_aug[:, :, HEAD_DIM], 1.0)
        nc.vector.memset(v_aug[:, :, VW + HEAD_DIM], 1.0)

        DR = mybir.MatmulPerfMode.DoubleRow
        NQK = 3 * NP                # DoubleRow steps per q/k projection

        def qk_step(wh_sb, wl_sb, b_sb, dst, sb, st, cell):
            """One fp8 DoubleRow step of the q or k projection of block sb.
            Steps 0..NP-1: wh*xh, NP..2NP-1: wh*xl, 2NP..3NP-1: wl*xh."""
            sl = slice(sb * QB, (sb + 1) * QB)
            term, tp = divmod(st, NP)
            w_t = (wh_sb, wh_sb, wl_sb)[term]
            x_t = (xh_sb, xl_sb, xh_sb)[term]
            if st == 0:
                cell["ps"] = aux_pool.tile([128, QB], F32, tag="aux",
                                           name="pps")
            nc.tensor.matmul(cell["ps"], lhsT=w_t[:, tp],
                             rhs=x_t[:, tp, :, sl], perf_mode=DR,
                             start=(st == 0), stop=(st == NQK - 1))
            if st == NQK - 1:
                nc.vector.tensor_scalar_add(dst[:, sl], cell["ps"], b_sb)

        def v_chunk(sb, c, cell):
            """V projection for token chunk c of block sb (fp8 DoubleRow,
            direct [token, head-dim] layout); evicts into v_aug."""
            tsl = slice(sb * QB + c * 128, sb * QB + (c + 1) * 128)
            if c == 0:
                cell["ps"] = aux_pool.tile([128, 4, 128], F32, tag="aux",
                                           name="vps")
            ps = cell["ps"]
            for st in range(NQK):
                term, tp = divmod(st, NP)
                x_t = (xh_sb, xl_sb, xh_sb)[term]
                w_t = (wvh_sb, wvh_sb, wvl_sb)[term]
                nc.tensor.matmul(ps[:, c, :], lhsT=x_t[:, tp, :, tsl],
                                 rhs=w_t[:, tp], perf_mode=DR,
                                 start=(st == 0), stop=(st == NQK - 1))
            kt_gl = sb * kt_per_qb + c
            # one strided copy fills both heads' v columns (strides 65/64)
            dst = v_aug[:, kt_gl, 0:2 * VW].rearrange(
                "p (a b) -> p a b", a=2, b=VW)[:, :, 0:HEAD_DIM]
            srcv = ps[:, c, :].rearrange("p (a b) -> p a b", a=2, b=HEAD_DIM)
            nc.vector.tensor_copy(dst, srcv)

        def proj_fillers(sb):
            units = []
            cq, ck, cv = {}, {}, {}
            for st2 in range(NQK // 2):
                units.append(lambda s=st2: (
                    qk_step(wqh_sb, wql_sb, bq_sb, qT_sb, sb, 2 * s, cq),
                    qk_step(wqh_sb, wql_sb, bq_sb, qT_sb, sb, 2 * s + 1, cq)))
            for st2 in range(NQK // 2):
                units.append(lambda s=st2: (
                    qk_step(wkh_sb, wkl_sb, bk_sb, kT_sb, sb, 2 * s, ck),
                    qk_step(wkh_sb, wkl_sb, bk_sb, kT_sb, sb, 2 * s + 1, ck)))
            for c in range(kt_per_qb):
                units.append(lambda c=c: v_chunk(sb, c, cv))
            return [("proj", sb, u) for u in units]

        ysb_cell = {}

        def yp_unit(b_i, j, at_bj, qt, u):
            """Out-projection: one [128 q, 512 e] matmul + evict; both
            halves land in one [128, 1024] staging tile, stored by a single
            DMA (halving the HWDGE dispatch load)."""
            at = at_bj[:, qt * 128:(qt + 1) * 128]
            yp = aux_pool.tile([128, QB], F32, tag="aux", name="yp")
            nc.tensor.matmul(yp, lhsT=at, rhs=wout_sb[:, u * QB:(u + 1) * QB],
                             start=True, stop=True)
            if u == 0:
                ysb_cell["t"] = ysb_pool.tile([128, N_EMBED], BF16,
                                              tag="ysb", name="ysb")
            ysb = ysb_cell["t"]
            # gpsimd cannot read PSUM: evictions go to DVE / ACT only
            if u == 0:
                nc.vector.tensor_copy(ysb[:, 0:QB], yp)
            else:
                nc.scalar.copy(ysb[:, QB:], yp)
                row0 = b_i * seq + j * QB + qt * 128
                nc.sync.dma_start(out=y.ap()[row0:row0 + 128, :], in_=ysb)

        def yp_fillers(b_i, j, at_bj):
            return [("yp", None,
                     lambda qt=qt, u=u: yp_unit(b_i, j, at_bj, qt, u))
                    for qt in range(QB // 128) for u in range(N_EMBED // QB)]

        fillers = collections.deque()

        def filler_tick(n, no_yp=False):
            """Emit up to n non-out-proj fillers plus at most one out-proj
            unit.  The out-proj unit rides on top of the budget: its ACT/DVE
            eviction copy delays the exp stream, so the iteration must
            stretch by extra PE work to keep the score pipeline fed."""
            popped_yp = 0
            for _ in range(min(n, len(fillers))):
                if fillers[0][0] == "yp":
                    if no_yp or popped_yp >= 1:
                        break
                    popped_yp += 1
                fillers.popleft()[2]()

        # ---- attention block ----
        def attn_block(b_i, j, f, late=(), eager=None, chunk_pv=False):
            """Software-pipelined k-loop for q-block j of batch b_i.

            Scores emitted diagonal-tiles-first (causal mask on DVE via the
            constant triangular mask); PV trails scores by 2 k-tiles; f
            fillers per iter; `late` fillers join the queue at iteration 2
            (previous block's out-proj, gated behind its norm chain)."""
            qsl = slice(b_i * seq + j * QB, b_i * seq + (j + 1) * QB)
            attn_ps = [attn_pool.tile([VW, QB], F32, tag=f"attn{h}",
                                      name=f"attn{h}") for h in range(H)]
            kts = list(range(kt_per_qb * j, kt_per_qb * (j + 1))) + \
                list(range(0, kt_per_qb * j))
            K = len(kts)
            saved = {}

            def emit_scores(kt):
                ks = slice(b_i * seq + kt * 128, b_i * seq + kt * 128 + 128)
                d = kt - kt_per_qb * j
                off = 128 * d if d >= 0 else 0
                s_ps = sy_pool.tile([128, H, QB], F32, tag="sy", name="s_ps")
                pt = pt_pool.tile([128, H, QB], BF16, tag="pt", name="pt")
                for h in range(H):
                    hsl = slice(HEAD_DIM * h, HEAD_DIM * (h + 1))
                    nc.tensor.matmul(
                        s_ps[:, h, off:],
                        lhsT=kT_sb[hsl, ks],
                        rhs=qT_sb[hsl, qsl.start + off:qsl.stop],
                        start=True, stop=True)
                nc.scalar.activation(pt[:, :, off:], s_ps[:, :, off:],
                                     AF.Exp, scale=SCALE / 4096.0)
                if d >= 0:  # diagonal: mask the 128 cols that need it
                    nc.vector.tensor_mul(pt[:, :, off:off + KT],
                                         pt[:, :, off:off + KT], cm_sb)
                saved[kt] = (pt, off)

            def emit_pv(kt, pos):
                if chunk_pv:
                    # j=0 block: accumulate column chunk #pos completely
                    # (k-tiles 0..pos back-to-back, a well-formed PSUM
                    # group) so it can normalize mid-epilogue
                    c = pos
                    for h in range(H):
                        for kt2 in range(c + 1):
                            pt, _ = saved[kt2]
                            nc.tensor.matmul(
                                attn_ps[h][:, c * 128:(c + 1) * 128],
                                lhsT=v_aug[:, b_i * n_kt + kt2,
                                           VW * h:VW * (h + 1)],
                                rhs=pt[:, h, c * 128:(c + 1) * 128],
                                start=(kt2 == 0), stop=(kt2 == c))
                    return
                pt, off = saved.pop(kt)
                for h in range(H):
                    nc.tensor.matmul(
                        attn_ps[h][:, off:],
                        lhsT=v_aug[:, b_i * n_kt + kt, VW * h:VW * (h + 1)],
                        rhs=pt[:, h, off:],
                        start=(pos == 0), stop=(pos == K - 1))

            filler_tick(3)
            emit_scores(kts[0])
            for t in range(K):
                if t + 1 < K:
                    emit_scores(kts[t + 1])
                if t == 2:
                    fillers.extend(late)
                filler_tick(f)
                if t - 2 >= 0:
                    emit_pv(kts[t - 2], t - 2)
                    if eager:
                        eager(attn_ps, t - 2)
            if K <= 2:
                fillers.extend(late)
            filler_tick(2)
            emit_pv(kts[K - 2], K - 2)
            if eager:
                eager(attn_ps, K - 2)
            filler_tick(2)
            emit_pv(kts[K - 1], K - 1)
            if eager:
                eager(attn_ps, K - 1)
            filler_tick(2)
            return attn_ps

        def norm_block(attn_ps):
            """Normalize the block's attention accumulators straight out of
            PSUM: reciprocal of the denominator row, broadcast, and a fused
            multiply-evict into the bf16 out-proj operand."""
            at_bj = at_pool.tile([DCORE, QB], BF16, name="at_bj")
            rfs, bcs = [], []
            for h in range(H):
                r0 = rec_pool.tile([1, QB], F32, tag=f"r0{h}", name=f"r0{h}")
                # x64 undoes the weight pre-scale carried by v through PV
                nc.vector.tensor_scalar_mul(
                    r0, attn_ps[h][HEAD_DIM:HEAD_DIM + 1, :], 64.0)
                rf = rec_pool.tile([1, QB], F32, tag=f"rf{h}", name=f"rf{h}")
                nc.vector.reciprocal_approx_fast(rf, r0)
                rfs.append(rf)
            for h in range(H):
                bc = bc_pool.tile([HEAD_DIM, QB], F32, tag=f"bc{h}",
                                  name=f"bc{h}")
                nc.gpsimd.partition_broadcast(bc, rfs[h])
                bcs.append(bc)
            for h in range(H):
                nc.vector.tensor_mul(
                    at_bj[HEAD_DIM * h:HEAD_DIM * (h + 1), :],
                    attn_ps[h][0:HEAD_DIM, :], bcs[h])
            return at_bj

        rot = [lambda o, i: nc.vector.tensor_copy(o, i),
               lambda o, i: nc.scalar.copy(o, i)]
        rot_i = [0]
        fin_cell = {"left": 0}

        def yp_unit_fin(at, row0, u):
            """Drain-phase out-proj unit: PSUM from the (now idle) score
            banks, eviction copies alternating DVE/ACT."""
            if fin_cell["left"] == 0:
                fin_cell["t"] = sy_pool.tile([128, H, QB], F32,
                                             tag="sy", name="ypw")
                fin_cell["left"] = H
            yp = fin_cell["t"][:, H - fin_cell["left"], :]
            fin_cell["left"] -= 1
            nc.tensor.matmul(yp, lhsT=at,
                             rhs=wout_sb[:, u * QB:(u + 1) * QB],
                             start=True, stop=True)
            if u == 0:
                fin_cell["ysb"] = ysb_pool.tile([128, N_EMBED], BF16,
                                                tag="ysb", name="ysb")
            ysb = fin_cell["ysb"]
            rot[rot_i[0] % 2](ysb[:, u * QB:(u + 1) * QB], yp)
            rot_i[0] += 1
            if u == 1:
                nc.sync.dma_start(out=y.ap()[row0:row0 + 128, :], in_=ysb)

        def norm_chunk(attn_ps, c, atqs):
            """Per-128-column normalize of a closed accumulator chunk."""
            csl = slice(c * 128, (c + 1) * 128)
            atq = at_pool.tile([DCORE, 128], BF16, tag="atq", name="atq")
            for h in range(H):
                r0 = rec_pool.tile([1, 128], F32, tag=f"r0c{h}",
                                   name=f"r0c{h}")
                nc.vector.tensor_scalar_mul(
                    r0, attn_ps[h][HEAD_DIM:HEAD_DIM + 1, csl], 64.0)
                rf = rec_pool.tile([1, 128], F32, tag=f"rfc{h}",
                                   name=f"rfc{h}")
                nc.vector.reciprocal_approx_fast(rf, r0)
                bc = bc_pool.tile([HEAD_DIM, 128], F32, tag=f"bcc{h}",
                                  name=f"bcc{h}")
                nc.gpsimd.partition_broadcast(bc, rf)
                nc.vector.tensor_mul(
                    atq[HEAD_DIM * h:HEAD_DIM * (h + 1), :],
                    attn_ps[h][0:HEAD_DIM, csl], bc)
            atqs[c] = atq

        # ---- schedule ----
        # attention order: small (b1, j0) last to shrink the tail
        order = [(0, 0), (0, 1), (0, 2), (0, 3),
                 (1, 1), (1, 2), (1, 3), (1, 0)]
        # proj groups to enqueue as fillers at each attention block start
        enqueue = {0: [1, 2], 1: [3], 2: [4], 3: [5], 4: [6], 5: [7]}

        # block 0 q/k projection is the critical path: emit directly,
        # q/k steps interleaved per d-tile pair to match the DMA feed rate
        cq0, ck0, cv0 = {}, {}, {}
        for st in range(NQK):
            qk_step(wqh_sb, wql_sb, bq_sb, qT_sb, 0, st, cq0)
            qk_step(wkh_sb, wkl_sb, bk_sb, kT_sb, 0, st, ck0)
        for c in range(kt_per_qb):
            fillers.append(("proj", 0, lambda c=c: v_chunk(0, c, cv0)))

        pending = None              # (b_i, j, attn_ps) awaiting norm + yp
        for idx, (b_i, j) in enumerate(order):
            for sb in enqueue.get(idx, []):
                fillers.extend(proj_fillers(sb))
            # fillers needed before the NEXT block can stream:
            # everything up to the last proj/v unit of its token block
            if idx + 1 < len(order):
                nb, nj = order[idx + 1]
                need_sb = {4 * nb + jj for jj in range(nj + 1)}
                needed = 0
                for i, (kind, sb, _) in enumerate(fillers):
                    if kind == "proj" and sb in need_sb:
                        needed = i + 1
            else:
                needed = len(fillers)
            K = 4 * (j + 1)
            f = max(4, math.ceil(needed / K))

            late = ()
            if pending is not None:
                at_prev = norm_block(pending[2])
                late = yp_fillers(pending[0], pending[1], at_prev)
            last = idx == len(order) - 1
            if last:
                # pipeline the last block's norm + out-proj inside its own
                # attention epilogue: chunk c closes at PV #c, normalizes
                # immediately, out-projects one PV later
                atqs = {}
                base = b_i * seq + j * QB

                def eager(aps, pos):
                    norm_chunk(aps, pos, atqs)
                    if pos >= 1:
                        for u in range(N_EMBED // QB):
                            yp_unit_fin(atqs[pos - 1],
                                        base + (pos - 1) * 128, u)
                attn_ps = attn_block(b_i, j, f, late, eager, chunk_pv=True)
                for u in range(N_EMBED // QB):
                    yp_unit_fin(atqs[kt_per_qb - 1],
                                base + (kt_per_qb - 1) * 128, u)
            else:
                attn_ps = attn_block(b_i, j, f, late)
            pending = (b_i, j, attn_ps)

        while fillers:
            filler_tick(1)

    nc.compile()
    return nc


_CACHE = {}


def _get_program(seq=S):
    if seq not in _CACHE:
        _CACHE[seq] = build_program(seq)
    return _CACHE[seq]


WSCALE = 64.0  # qkv weight pre-scale: clears the e4m3 subnormal floor


def make_in_maps(x, W_qkv, b_qkv, seq=S):
    bf16 = ml_dtypes.bfloat16
    fp8 = ml_dtypes.float8_e4m3fn
    s_tot = B * seq
    NP = DT // 2

    # x split hi/lo in e4m3, laid out [128, NP, 2, s_tot]:
    # embed index e = tp*256 + pair*128 + p
    xf = x.reshape(s_tot, N_EMBED)
    x_hi = xf.astype(fp8)
    x_lo = (xf - x_hi.astype(np.float32)).astype(fp8)

    def xarr(xv):
        return np.ascontiguousarray(
            xv.T.reshape(NP, 2, 128, s_tot).transpose(2, 0, 1, 3))

    def wsplit(w):                    # [1024, DCORE] -> hi/lo [128,NP,2,DCORE]
        ws = w * WSCALE
        wh = ws.astype(fp8)
        wl = (ws - wh.astype(np.float32)).astype(fp8)

        def arr(wv):
            return np.ascontiguousarray(
                wv.reshape(NP, 2, 128, DCORE).transpose(2, 0, 1, 3))
        return arr(wh), arr(wl)

    # causal mask for diagonal k-tiles (same for both heads)
    cmask = (np.arange(KT)[None, :] >= np.arange(128)[:, None])
    cmask = np.ascontiguousarray(
        np.broadcast_to(cmask[:, None, :], (128, H, KT))).astype(bf16)

    xh_a, xl_a = xarr(x_hi), xarr(x_lo)
    in_maps = []
    for c in range(N_CORES):
        csl = slice(DCORE * c, DCORE * (c + 1))
        wqh_a, wql_a = wsplit(W_qkv[:, csl])
        wkh_a, wkl_a = wsplit(W_qkv[:, N_EMBED:][:, csl])
        wvh_a, wvl_a = wsplit(W_qkv[:, 2 * N_EMBED:][:, csl])
        in_maps.append({
            "xh": xh_a, "xl": xl_a,
            "wqh": wqh_a, "wql": wql_a,
            "wkh": wkh_a, "wkl": wkl_a,
            "wvh": wvh_a, "wvl": wvl_a,
            "bq": np.ascontiguousarray(
                (b_qkv[csl] * WSCALE).reshape(DCORE, 1)).astype(np.float32),
            "bk": np.ascontiguousarray(
                (b_qkv[N_EMBED:][csl] * WSCALE)
                .reshape(DCORE, 1)).astype(np.float32),
            "cm": cmask,
            "wout": None,  # filled by caller
        })
    return in_maps


def kernel(x, W_qkv, b_qkv, W_out, b_out):
    x = np.asarray(x, dtype=np.float32)
    W_qkv = np.asarray(W_qkv, dtype=np.float32)
    b_qkv = np.asarray(b_qkv, dtype=np.float32)
    W_out = np.asarray(W_out, dtype=np.float32)
    b_out = np.asarray(b_out, dtype=np.float32)

    nc = _get_program(S)
    in_maps = make_in_maps(x, W_qkv, b_qkv, S)
    bf16 = ml_dtypes.bfloat16
    for c in range(N_CORES):
        csl = slice(DCORE * c, DCORE * (c + 1))
        in_maps[c]["wout"] = np.ascontiguousarray(W_out[csl, :]).astype(bf16)

    res = run_bass_kernel_spmd(nc, in_maps, core_ids=list(range(N_CORES)))
    y = np.zeros((B * S, N_EMBED), dtype=np.float32)
    for r in res.results:
        y += r["y"].astype(np.float32)
    # bias + v-bias folded through W_out (softmax rows sum to 1)
    y += b_out[None, :] + b_qkv[2 * N_EMBED:] @ W_out
    return y.reshape(B, S, N_EMBED)
